# revision 2
# baseline (speedup 1.0000x reference)
"""Causal multi-head attention on 8 TRN2 NeuronCores.

Problem: q,k,v [4, 16, 2048, 64] f32 -> out [4, 16, 2048, 64] f32
  out = softmax(causal(Q K^T / sqrt(64))) V  per (batch, head)

Sharding: 64 (b,h) pairs are split across 8 cores (8 pairs per core), no
cross-core communication.

Per-core algorithm per (b,h) pair (S=2048, D=64, P=128):
  - Load Q,K,V tiles, cast to bf16.
  - PE-transpose Q,K into [D, S] layout (contraction over D needs D on
    partitions).
  - For each key-tile jb (16 of them), compute the transposed score block
    ST[j, i] = K Q^T for i >= jb*128 only (causal), exp it on the Scalar
    engine (PSUM -> SBUF bf16, scale fused), mask the diagonal tile, and
    accumulate O^T[c, i] = sum_j [V | 1][j, c] * E[j, i] on the PE into a
    PSUM accumulator.  The appended ones-column makes row 64 of O^T the
    softmax denominator.
  - PE-transpose O^T back to [i, c] tiles, multiply by the reciprocal of
    the denominator, and DMA out.
"""
import sys

if '/opt/trn_rl_repo' not in sys.path:
    sys.path.insert(0, '/opt/trn_rl_repo')

import numpy as np

import concourse.bacc as bacc
import concourse.mybir as mybir
import concourse.tile as tile
from concourse import masks

B, H, S, D = 4, 16, 2048, 64
N_CORES = 8
BH_PER_CORE = (B * H) // N_CORES  # 8
SCALE = float(D) ** -0.5
P = 128
NT = S // P  # 16 key/query tiles per (b,h)
BF = mybir.dt.bfloat16
F32 = mybir.dt.float32


def _units(jb):
    """exp units for key-tile jb: absolute i-ranges, each <= 1024 wide."""
    i0 = jb * P
    if i0 < 1024:
        return [(i0, 1024), (1024, 2048)]
    return [(i0, 2048)]


def _chunks(a, b, grid):
    """split [a, b) at multiples of `grid`."""
    out = []
    while a < b:
        nxt = min(b, (a // grid + 1) * grid)
        out.append((a, nxt))
        a = nxt
    return out


def build_nc():
    nc = bacc.Bacc()
    q_ext = nc.declare_dram_parameter("q", [BH_PER_CORE, S, D], F32, isOutput=False)
    k_ext = nc.declare_dram_parameter("k", [BH_PER_CORE, S, D], F32, isOutput=False)
    v_ext = nc.declare_dram_parameter("v", [BH_PER_CORE, S, D], F32, isOutput=False)
    out_ext = nc.declare_dram_parameter("out", [BH_PER_CORE, S, D], F32, isOutput=True)

    with tile.TileContext(nc) as tc:
        with (
            tc.tile_pool(name="const", bufs=1) as const_pool,
            tc.tile_pool(name="io", bufs=2) as io_pool,
            tc.tile_pool(name="bfp", bufs=2) as bf_pool,
            tc.tile_pool(name="tr", bufs=2) as tr_pool,
            tc.tile_pool(name="ep", bufs=4) as e_pool,
            tc.tile_pool(name="epi", bufs=2) as epi_pool,
            tc.tile_pool(name="ps", bufs=2, space="PSUM") as ps_pool,
            tc.tile_pool(name="po", bufs=1, space="PSUM") as po_pool,
        ):
            ident = const_pool.tile([P, P], BF)
            masks.make_identity(nc, ident)
            # keep-mask for the diagonal score tile: 1 where j_local <= i_local
            trimask = const_pool.tile([P, P], BF)
            masks.make_upper_triangular(nc, trimask, val=1.0, diag=True)

            for bh in range(BH_PER_CORE):
                # ---- load + cast ----
                q_sb = io_pool.tile([P, NT, D], F32, tag="q_sb")
                k_sb = io_pool.tile([P, NT, D], F32, tag="k_sb")
                v_sb = io_pool.tile([P, NT, D], F32, tag="v_sb")
                nc.gpsimd.dma_start(
                    out=q_sb, in_=q_ext[bh].rearrange("(t p) d -> p t d", p=P))
                nc.gpsimd.dma_start(
                    out=k_sb, in_=k_ext[bh].rearrange("(t p) d -> p t d", p=P))
                nc.gpsimd.dma_start(
                    out=v_sb, in_=v_ext[bh].rearrange("(t p) d -> p t d", p=P))

                q_bf = bf_pool.tile([P, NT, D], BF, tag="q_bf")
                k_bf = bf_pool.tile([P, NT, D], BF, tag="k_bf")
                vp = bf_pool.tile([P, NT, D + 1], BF, tag="vp")
                nc.vector.tensor_copy(q_bf, q_sb)
                nc.vector.tensor_copy(k_bf, k_sb)
                nc.vector.tensor_copy(vp[:, :, :D], v_sb)
                nc.gpsimd.memset(vp[:, :, D:], 1.0)

                # ---- transpose Q,K -> [D, S] bf16 ----
                qt = tr_pool.tile([D, S], BF, tag="qt")
                kt = tr_pool.tile([D, S], BF, tag="kt")
                for g in range(NT // 4):  # 4 tiles per psum buffer
                    q_tp = ps_pool.tile([D, 4 * P], BF, tag="st")
                    k_tp = ps_pool.tile([D, 4 * P], BF, tag="st")
                    for u in range(4):
                        t = g * 4 + u
                        nc.tensor.transpose(
                            q_tp[:, u * P:(u + 1) * P], q_bf[:, t, :], ident)
                        nc.tensor.transpose(
                            k_tp[:, u * P:(u + 1) * P], k_bf[:, t, :], ident)
                    nc.vector.tensor_copy(qt[:, g * 4 * P:(g + 1) * 4 * P], q_tp)
                    nc.vector.tensor_copy(kt[:, g * 4 * P:(g + 1) * 4 * P], k_tp)

                # ---- main loop: scores^T -> exp -> O^T accumulation ----
                ot = po_pool.tile([D + 1, S], F32, tag="ot")
                for jb in range(NT):
                    i0 = jb * P
                    kt_j = kt[:, i0:i0 + P]  # [64, 128] stationary
                    for (u0, u1) in _units(jb):
                        w = u1 - u0
                        st = ps_pool.tile([P, 1024], F32, tag="st")
                        for (a, b) in _chunks(0, w, 512):
                            nc.tensor.matmul(
                                st[:, a:b], kt_j, qt[:, u0 + a:u0 + b])
                        e_sb = e_pool.tile([P, 1024], BF, tag="e")
                        nc.scalar.activation(
                            out=e_sb[:, :w], in_=st[:, :w],
                            func=mybir.ActivationFunctionType.Exp, scale=SCALE)
                        if u0 == i0:  # diagonal tile: causal mask
                            nc.vector.tensor_mul(
                                e_sb[:, :P], e_sb[:, :P], trimask)
                        for (a, b) in _chunks(u0, u1, 512):
                            ci = a // 512
                            nc.tensor.matmul(
                                ot[:, a:b], vp[:, jb, :], e_sb[:, a - u0:b - u0],
                                start=(jb == 0), stop=(jb == 4 * ci + 3))

                # ---- epilogue: copy O^T, transpose back, normalize ----
                ot_sb = epi_pool.tile([D + 1, S], BF, tag="ot_sb")
                nc.vector.tensor_copy(ot_sb, ot)
                o_sb = epi_pool.tile([P, NT, D], F32, tag="o_sb")
                rcp = epi_pool.tile([P, NT], F32, tag="rcp")
                for g in range(NT // 4):
                    # inner dim padded to 66 so each [128, 65] transpose lands
                    # 4-byte aligned in PSUM (66 * 2B = 132B, multiple of 4)
                    tp = ps_pool.tile([P, 4, D + 2], BF, tag="st")
                    for u in range(4):
                        t = g * 4 + u
                        nc.tensor.transpose(
                            tp[:, u, :D + 1], ot_sb[:, t * P:(t + 1) * P],
                            ident[:D + 1, :D + 1])
                    nc.vector.reciprocal(
                        out=rcp[:, g * 4:(g + 1) * 4], in_=tp[:, :, D])
                    for u in range(4):
                        t = g * 4 + u
                        nc.vector.tensor_scalar_mul(
                            o_sb[:, t, :], tp[:, u, :D], rcp[:, t:t + 1])
                nc.gpsimd.dma_start(
                    out=out_ext[bh].rearrange("(t p) d -> p t d", p=P), in_=o_sb)

    nc.compile()
    return nc


_CACHE = {}


def _get_runner():
    """Build + compile once; return a cached jitted 8-core runner."""
    if "runner" in _CACHE:
        return _CACHE["runner"]

    import jax
    from jax.sharding import Mesh, PartitionSpec
    from jax.experimental.shard_map import shard_map
    from concourse import bass2jax
    from concourse.bass2jax import _bass_exec_p, partition_id_tensor
    import concourse.mybir as _mybir

    nc = build_nc()
    bass2jax.install_neuronx_cc_hook()

    partition_name = nc.partition_id_tensor.name if nc.partition_id_tensor else None
    in_names, out_names, out_avals = [], [], []
    for alloc in nc.m.functions[0].allocations:
        if not isinstance(alloc, _mybir.MemoryLocationSet):
            continue
        name = alloc.memorylocations[0].name
        if alloc.kind == "ExternalInput":
            if name != partition_name:
                in_names.append(name)
        elif alloc.kind == "ExternalOutput":
            shape = tuple(alloc.tensor_shape)
            dtype = _mybir.dt.np(alloc.dtype)
            out_names.append(name)
            out_avals.append(jax.core.ShapedArray(shape, dtype))
    n_params = len(in_names)
    all_names = list(in_names) + list(out_names)
    if partition_name is not None:
        all_names.append(partition_name)

    def _body(*args):
        operands = list(args)
        if partition_name is not None:
            operands.append(partition_id_tensor())
        outs = _bass_exec_p.bind(
            *operands,
            out_avals=tuple(out_avals),
            in_names=tuple(all_names),
            out_names=tuple(out_names),
            lowering_input_output_aliases=(),
            sim_require_finite=True,
            sim_require_nnan=True,
            nc=nc,
        )
        return tuple(outs)

    devices = jax.devices()[:N_CORES]
    mesh = Mesh(np.asarray(devices), ("core",))
    n_outs = len(out_names)
    in_specs = (PartitionSpec("core"),) * (n_params + n_outs)
    out_specs = (PartitionSpec("core"),) * n_outs
    sharded = jax.jit(shard_map(
        _body, mesh=mesh, in_specs=in_specs, out_specs=out_specs,
        check_rep=False))

    runner = {
        "fn": sharded,
        "in_names": in_names,
        "out_names": out_names,
        "out_avals": out_avals,
        "mesh": mesh,
    }
    _CACHE["runner"] = runner
    return runner


def _shard(x):
    """[B, H, S, D] -> concatenated per-core [(N_CORES*BH_PER_CORE), S, D]."""
    return np.ascontiguousarray(x.reshape(B * H, S, D))


def kernel(q, k, v):
    q = np.asarray(q, dtype=np.float32)
    k = np.asarray(k, dtype=np.float32)
    v = np.asarray(v, dtype=np.float32)
    r = _get_runner()
    ins = {"q": _shard(q), "k": _shard(k), "v": _shard(v)}
    concat_in = [ins[name] for name in r["in_names"]]
    zeros = [np.zeros((N_CORES * av.shape[0],) + av.shape[1:], av.dtype)
             for av in r["out_avals"]]
    outs = r["fn"](*concat_in, *zeros)
    out = np.asarray(outs[r["out_names"].index("out")])
    return out.reshape(B, H, S, D)


# revision 29
# speedup vs baseline: 49.3036x; 49.3036x over previous
"""Causal multi-head attention on 8 TRN2 NeuronCores.

Problem: q,k,v [4, 16, 2048, 64] f32 -> out [4, 16, 2048, 64] f32
  out = softmax(causal(Q K^T / sqrt(64))) V  per (batch, head)

Sharding: 64 (b,h) pairs are split across 8 cores (8 pairs per core), no
cross-core communication.

Per-core algorithm per (b,h) pair (S=2048, D=64, P=128):
  - Load Q,K,V tiles, cast to fp16 (gpsimd).
  - PE-transpose Q,K into [D, S] layout (contraction over D needs D on
    partitions).
  - Loop i-chunks (1024 queries) then key-tiles jb: compute the transposed
    score block ST[j, i] = K Q^T for the causal i >= jb*128 range, exp it on
    the Scalar engine (PSUM -> SBUF fp16, scale fused), mask the diagonal
    tile, and accumulate O^T[c, i] = sum_j [V | 1][j, c] * E[j, i] on the PE
    into a PSUM accumulator.  The ones-column makes row 64 of O^T the softmax
    denominator.  Emission is software-pipelined: the next unit's score
    matmuls are issued before the current unit's PV matmuls so the PE never
    waits on the Scalar engine.
  - Per i-chunk: PE-transpose O^T back to [i, c] tiles, multiply by the
    reciprocal of the denominator, DMA out.
"""
import sys

if '/opt/trn_rl_repo' not in sys.path:
    sys.path.insert(0, '/opt/trn_rl_repo')

import os

import numpy as np

import concourse.bacc as bacc
import concourse.bass as bass
import concourse.mybir as mybir
import concourse.tile as tile
from concourse import masks

B, H, S, D = 4, 16, 2048, 64
N_CORES = 8
BH_PER_CORE = (B * H) // N_CORES  # 8
SCALE = float(D) ** -0.5
P = 128
NT = S // P  # 16 key/query tiles per (b,h)
CW = 1024    # i-chunk width
NC_CHUNK = S // CW  # 2
F16 = mybir.dt.float16
F32 = mybir.dt.float32

# tuning knobs (env-overridable for experiments; defaults = tuned values)
# K_LAYOUT: A = prep+epi psum tiles share the "st" slots (ST_BUFS=3)
#           B = epi tiles share the "ot" slot, prep shares "st"
#           C = prep+epi in their own 2-slot pool, ST_BUFS=2
LAYOUT = os.environ.get("K_LAYOUT", "C")
ST_BUFS = int(os.environ.get("K_ST_BUFS", "2" if LAYOUT == "C" else "3"))
E_BUFS = int(os.environ.get("K_E_BUFS", "8"))
PIPE_DEPTH = int(os.environ.get("K_PIPE_DEPTH", "2"))
# K_OFFBIG exp units per (b,h) are computed on the DVE via the Schraudolph
# bit-trick in fp16 space (one tensor_scalar producing int16 exponent bits,
# bitcast to fp16) instead of the Scalar engine, balancing the two engines.
OFF_BIG = int(os.environ.get("K_OFFBIG", "6"))
# fp16-space Schraudolph: bits16 = int16(x*A16 + B16) viewed as fp16
SCH_A16 = float(2**10 / np.log(2)) * SCALE
SCH_B16 = float(15 * 2**10) - 60.0
PREP_AT = int(os.environ.get("K_PREP_AT", "23"))


def _chunks(a, b, grid):
    out = []
    while a < b:
        nxt = min(b, (a // grid + 1) * grid)
        out.append((a, nxt))
        a = nxt
    return out


def build_nc():
    nc = bacc.Bacc()
    q_ext = nc.declare_dram_parameter("q", [BH_PER_CORE, S, D], F32, isOutput=False)
    k_ext = nc.declare_dram_parameter("k", [BH_PER_CORE, S, D], F32, isOutput=False)
    v_ext = nc.declare_dram_parameter("v", [BH_PER_CORE, S, D], F32, isOutput=False)
    out_ext = nc.declare_dram_parameter("out", [BH_PER_CORE, S, D], F32, isOutput=True)

    with tile.TileContext(nc) as tc:
        with (
            tc.tile_pool(name="const", bufs=1) as const_pool,
            tc.tile_pool(name="io", bufs=2) as io_pool,
            tc.tile_pool(name="bfp", bufs=2) as bf_pool,
            tc.tile_pool(name="tr", bufs=2) as tr_pool,
            tc.tile_pool(name="ep", bufs=E_BUFS) as e_pool,
            tc.tile_pool(name="epi", bufs=2) as epi_pool,
            tc.tile_pool(name="ps", bufs=ST_BUFS, space="PSUM") as ps_pool,
            tc.tile_pool(name="po", bufs=1, space="PSUM") as po_pool,
            tc.tile_pool(name="pp", bufs=2, space="PSUM") as pp_pool,
        ):
            if LAYOUT == "A":
                prep_alloc = lambda shape, dt: ps_pool.tile(shape, dt, tag="st", name="prep")
                tp_alloc = lambda shape, dt: ps_pool.tile(shape, dt, tag="st", name="tp")
            elif LAYOUT == "B":
                prep_alloc = lambda shape, dt: ps_pool.tile(shape, dt, tag="st", name="prep")
                tp_alloc = lambda shape, dt: po_pool.tile(shape, dt, tag="ot", name="tp")
            else:
                prep_alloc = lambda shape, dt: pp_pool.tile(shape, dt, tag="pp", name="prep")
                tp_alloc = lambda shape, dt: pp_pool.tile(shape, dt, tag="pp", name="tp")
            ident = const_pool.tile([P, P], F16)
            masks.make_identity(nc, ident)
            ident32 = const_pool.tile([D + 1, D + 1], F32)
            masks.make_identity(nc, ident32)
            # touch Exp once so the ACT table load overlaps the first DMAs
            warm = const_pool.tile([P, 1], F32)
            nc.vector.memset(warm, 0.0)
            nc.scalar.activation(out=warm, in_=warm,
                                 func=mybir.ActivationFunctionType.Exp)
            # keep-mask for the diagonal score tile: 1 where j_local <= i_local
            trimask = const_pool.tile([P, P], F16)
            masks.make_upper_triangular(nc, trimask, val=1.0, diag=True)

            def prep(bh):
                """Load Q,K,V; cast to fp16; PE-transpose Q,K.

                qt: [128, S] with QT duplicated on both partition halves;
                kt: [128, NT/2, 128] with even key-tiles on partitions 0-63
                and odd tiles on 64-127.  Adjacent jb score matmuls then hit
                different PE row-groups and run concurrently on hardware.
                Work is pipelined in 4-tile groups so casts/transposes start
                before the full tensors arrive.
                """
                qt = tr_pool.tile([P, S], F16, tag="qt", name="qt")
                kt = tr_pool.tile([P, NT // 2, P], F16, tag="kt", name="kt")
                q_view = q_ext[bh].rearrange("(t p) d -> p t d", p=P)
                k_view = k_ext[bh].rearrange("(t p) d -> p t d", p=P)
                for g in range(NT // 8):
                    k_sb = io_pool.tile([P, 8, D], F32, tag="k_sb", name="k_sb")
                    nc.sync.dma_start(out=k_sb, in_=k_view[:, 8 * g:8 * g + 8, :])
                    k_bf = bf_pool.tile([P, 8, D], F16, tag="k_bf", name="k_bf")
                    nc.gpsimd.tensor_copy(k_bf, k_sb)
                    k_tp = prep_alloc([P, 4 * P], F16)
                    for u in range(4):
                        nc.tensor.transpose(
                            k_tp[:, u * P:(u + 1) * P],
                            k_bf[:, 2 * u:2 * u + 2, :], ident)
                    nc.vector.tensor_copy(kt[:, g * 4:(g + 1) * 4, :], k_tp)
                for g in range(NT // 4):
                    q_sb = io_pool.tile([P, 4, D], F32, tag="q_sb", name="q_sb")
                    nc.sync.dma_start(out=q_sb, in_=q_view[:, 4 * g:4 * g + 4, :])
                    q_bf = bf_pool.tile([P, 4, D], F16, tag="q_bf", name="q_bf")
                    nc.gpsimd.tensor_copy(q_bf, q_sb)
                    q_tp = prep_alloc([D, 4 * P], F16)
                    for u in range(4):
                        nc.tensor.transpose(
                            q_tp[:, u * P:(u + 1) * P], q_bf[:, u, :], ident)
                    sl = slice(g * 4 * P, (g + 1) * 4 * P)
                    nc.vector.tensor_copy(qt[:D, sl], q_tp)
                    # duplicate onto partitions 64-127 for row-group packing
                    # (gpsimd cannot read PSUM, so copy from the SBUF half)
                    nc.gpsimd.tensor_copy(qt[D:, sl], qt[:D, sl])
                v_sb = io_pool.tile([P, NT, D], F32, tag="v_sb", name="v_sb")
                nc.sync.dma_start(
                    out=v_sb, in_=v_ext[bh].rearrange("(t p) d -> p t d", p=P))
                vp = bf_pool.tile([P, NT, D + 1], F16, tag="vp", name="vp")
                nc.gpsimd.tensor_copy(vp[:, :, :D], v_sb)
                nc.gpsimd.memset(vp[:, :, D:], 1.0)
                return qt, kt, vp

            preps = {0: prep(0)}
            for bh in range(BH_PER_CORE):
                qt, kt, vp = preps.pop(bh)
                # ---- main loop: i-chunk major, software-pipelined units ----
                # unit = (ci, jb): score block ST[j, i] for
                #   i in [max(ci*CW, jb*P), (ci+1)*CW), j in [jb*P, (jb+1)*P)
                units = []
                for ci in range(NC_CHUNK):
                    c0 = ci * CW
                    for jb in range(((ci + 1) * CW) // P):
                        units.append((ci, jb, max(c0, jb * P), (ci + 1) * CW))

                ots = {}     # ci -> psum accumulator [D+1, CW]
                stage = []   # pipelined: [(unit, st_tile, e_tile), ...]

                # spread the DVE-offloaded units over units wide enough that
                # their PV chunks stay >= 256 (f32r full-rate)
                # offloadable: wide enough for full-rate PV, and never a
                # dominant share of any query row's softmax mass (rows in
                # [jb*P, ...) get 1/(jb+1) of their mass from key-tile jb)
                cands = [i for i, (ci_, jb_, u0_, u1_) in enumerate(units)
                         if u1_ - u0_ >= 384 and (ci_ == 1 or jb_ >= 3)]
                _soff = int(os.environ.get("K_OFF_SHIFT", "1"))
                off_set = set(
                    cands[(round(i * len(cands) / OFF_BIG) + _soff) % len(cands)]
                    for i in range(OFF_BIG)) if OFF_BIG else set()

                def flush_pv(ci, jb, u0, u1, e_sb):
                    c0 = ci * CW
                    for (a, b) in _chunks(u0, u1, 512):
                        # last key-tile writing this 512-wide psum cell
                        cell0 = (a // 512) * 512
                        cell_last = (cell0 + 511) // P
                        nc.tensor.matmul(
                            ots[ci][:, a - c0:b - c0], vp[:, jb, :],
                            e_sb[:, a - u0:b - u0],
                            start=(jb == 0), stop=(jb == cell_last))

                for uidx, (ci, jb, u0, u1) in enumerate(units):
                    if jb == 0:
                        ots[ci] = po_pool.tile([D + 1, CW], F32, tag="ot", name=f"ot{ci}")
                    w = u1 - u0
                    st = ps_pool.tile([P, CW], F32, tag="st")
                    half = (jb % 2) * D
                    for (a, b) in _chunks(0, w, 512):
                        nc.tensor.matmul(
                            st[:, a:b], kt[half:half + D, jb // 2, :],
                            qt[half:half + D, u0 + a:u0 + b])
                    if uidx in off_set:
                        ei = e_pool.tile([P, CW], mybir.dt.int16, tag="e",
                                         name="ei")
                        nc.vector.tensor_scalar(
                            out=ei[:, :w], in0=st[:, :w],
                            scalar1=SCH_A16, scalar2=SCH_B16,
                            op0=mybir.AluOpType.mult, op1=mybir.AluOpType.add)
                        e_sb = ei.bitcast(F16)
                    else:
                        e_sb = e_pool.tile([P, CW], F16, tag="e")
                        nc.scalar.activation(
                            out=e_sb[:, :w], in_=st[:, :w],
                            func=mybir.ActivationFunctionType.Exp, scale=SCALE)
                    if u0 == jb * P:  # diagonal tile: causal mask
                        nc.vector.tensor_mul(
                            e_sb[:, :P], e_sb[:, :P], trimask)
                    stage.append((ci, jb, u0, u1, e_sb))
                    if len(stage) > PIPE_DEPTH:
                        flush_pv(*stage.pop(0))
                    if uidx == PREP_AT and bh + 1 < BH_PER_CORE:
                        preps[bh + 1] = prep(bh + 1)
                    if jb == ((ci + 1) * CW) // P - 1:
                        # drain so the epilogue can read this chunk's ot
                        while stage:
                            flush_pv(*stage.pop(0))
                        _epilogue(nc, ci, ots.pop(ci), epi_pool, tp_alloc,
                                  ident32, out_ext, bh)

    nc.compile()
    return nc


def _epilogue(nc, ci, ot, epi_pool, tp_alloc, ident32, out_ext, bh):
    """Copy O^T out of PSUM, transpose back to [i, c], normalize, DMA out."""
    c0 = ci * CW
    ntile = CW // P  # 8
    ot_sb = epi_pool.tile([D + 1, CW], F32, tag="ot_sb")
    nc.vector.tensor_copy(ot_sb, ot)
    o_sb = epi_pool.tile([P, ntile, D], F32, tag="o_sb")
    rcp = epi_pool.tile([P, ntile], F32, tag="rcp")
    for g in range(ntile // 4):
        # inner dim padded to 66 to keep per-transpose offsets regular
        tp = tp_alloc([P, 4, D + 2], F32)
        for u in range(4):
            t = g * 4 + u
            nc.tensor.transpose(
                tp[:, u, :D + 1], ot_sb[:, t * P:(t + 1) * P], ident32)
        nc.vector.reciprocal(out=rcp[:, g * 4:(g + 1) * 4], in_=tp[:, :, D])
        rsl = rcp[:, g * 4:(g + 1) * 4]
        rcp_b = bass.AP(tensor=rsl.tensor, offset=rsl.offset,
                        ap=[rsl.ap[0], rsl.ap[1], [0, D]])
        nc.vector.tensor_tensor(
            out=o_sb[:, g * 4:(g + 1) * 4, :], in0=tp[:, :, :D], in1=rcp_b,
            op=mybir.AluOpType.mult)
    nc.sync.dma_start(
        out=out_ext[bh, c0:c0 + CW].rearrange("(t p) d -> p t d", p=P),
        in_=o_sb)


_CACHE = {}


def _get_runner():
    """Build + compile once; return a cached jitted 8-core runner."""
    if "runner" in _CACHE:
        return _CACHE["runner"]

    import jax
    from jax.sharding import Mesh, PartitionSpec
    from jax.experimental.shard_map import shard_map
    from concourse import bass2jax
    from concourse.bass2jax import _bass_exec_p, partition_id_tensor
    import concourse.mybir as _mybir

    nc = build_nc()
    bass2jax.install_neuronx_cc_hook()

    partition_name = nc.partition_id_tensor.name if nc.partition_id_tensor else None
    in_names, out_names, out_avals = [], [], []
    for alloc in nc.m.functions[0].allocations:
        if not isinstance(alloc, _mybir.MemoryLocationSet):
            continue
        name = alloc.memorylocations[0].name
        if alloc.kind == "ExternalInput":
            if name != partition_name:
                in_names.append(name)
        elif alloc.kind == "ExternalOutput":
            shape = tuple(alloc.tensor_shape)
            dtype = _mybir.dt.np(alloc.dtype)
            out_names.append(name)
            out_avals.append(jax.core.ShapedArray(shape, dtype))
    n_params = len(in_names)
    all_names = list(in_names) + list(out_names)
    if partition_name is not None:
        all_names.append(partition_name)

    def _body(*args):
        operands = list(args)
        if partition_name is not None:
            operands.append(partition_id_tensor())
        outs = _bass_exec_p.bind(
            *operands,
            out_avals=tuple(out_avals),
            in_names=tuple(all_names),
            out_names=tuple(out_names),
            lowering_input_output_aliases=(),
            sim_require_finite=True,
            sim_require_nnan=True,
            nc=nc,
        )
        return tuple(outs)

    devices = jax.devices()[:N_CORES]
    mesh = Mesh(np.asarray(devices), ("core",))
    n_outs = len(out_names)
    in_specs = (PartitionSpec("core"),) * (n_params + n_outs)
    out_specs = (PartitionSpec("core"),) * n_outs
    sharded = jax.jit(shard_map(
        _body, mesh=mesh, in_specs=in_specs, out_specs=out_specs,
        check_rep=False))

    runner = {
        "fn": sharded,
        "in_names": in_names,
        "out_names": out_names,
        "out_avals": out_avals,
        "mesh": mesh,
    }
    _CACHE["runner"] = runner
    return runner


def _shard(x):
    """[B, H, S, D] -> concatenated per-core [(N_CORES*BH_PER_CORE), S, D]."""
    return np.ascontiguousarray(x.reshape(B * H, S, D))


def kernel(q, k, v):
    q = np.asarray(q, dtype=np.float32)
    k = np.asarray(k, dtype=np.float32)
    v = np.asarray(v, dtype=np.float32)
    r = _get_runner()
    ins = {"q": _shard(q), "k": _shard(k), "v": _shard(v)}
    concat_in = [ins[name] for name in r["in_names"]]
    zeros = [np.zeros((N_CORES * av.shape[0],) + av.shape[1:], av.dtype)
             for av in r["out_avals"]]
    outs = r["fn"](*concat_in, *zeros)
    out = np.asarray(outs[r["out_names"].index("out")])
    return out.reshape(B, H, S, D)



# revision 42
# speedup vs baseline: 50.6598x; 1.0275x over previous
"""Causal multi-head attention on 8 TRN2 NeuronCores.

Problem: q,k,v [4, 16, 2048, 64] f32 -> out [4, 16, 2048, 64] f32
  out = softmax(causal(Q K^T / sqrt(64))) V  per (batch, head)

Sharding: 64 (b,h) pairs are split across 8 cores (8 pairs per core), no
cross-core communication.

Per-core algorithm per (b,h) pair (S=2048, D=64, P=128):
  - Load Q,K,V tiles, cast to fp16 (gpsimd).
  - PE-transpose Q,K into [D, S] layout (contraction over D needs D on
    partitions).
  - Loop i-chunks (1024 queries) then key-tiles jb: compute the transposed
    score block ST[j, i] = K Q^T for the causal i >= jb*128 range, exp it on
    the Scalar engine (PSUM -> SBUF fp16, scale fused), mask the diagonal
    tile, and accumulate O^T[c, i] = sum_j [V | 1][j, c] * E[j, i] on the PE
    into a PSUM accumulator.  The ones-column makes row 64 of O^T the softmax
    denominator.  Emission is software-pipelined: the next unit's score
    matmuls are issued before the current unit's PV matmuls so the PE never
    waits on the Scalar engine.
  - Per i-chunk: PE-transpose O^T back to [i, c] tiles, multiply by the
    reciprocal of the denominator, DMA out.
"""
import sys

if '/opt/trn_rl_repo' not in sys.path:
    sys.path.insert(0, '/opt/trn_rl_repo')

import os

import numpy as np

import concourse.bacc as bacc
import concourse.bass as bass
import concourse.mybir as mybir
import concourse.tile as tile
from concourse import masks

B, H, S, D = 4, 16, 2048, 64
N_CORES = 8
BH_PER_CORE = (B * H) // N_CORES  # 8
SCALE = float(D) ** -0.5
P = 128
NT = S // P  # 16 key/query tiles per (b,h)
CW = 1024    # i-chunk width
NC_CHUNK = S // CW  # 2
F16 = mybir.dt.float16
F32 = mybir.dt.float32

# tuning knobs (env-overridable for experiments; defaults = tuned values)
# K_LAYOUT: A = prep+epi psum tiles share the "st" slots (ST_BUFS=3)
#           B = epi tiles share the "ot" slot, prep shares "st"
#           C = prep+epi in their own 2-slot pool, ST_BUFS=2
LAYOUT = os.environ.get("K_LAYOUT", "C")
ST_BUFS = int(os.environ.get("K_ST_BUFS", "2" if LAYOUT == "C" else "3"))
E_BUFS = int(os.environ.get("K_E_BUFS", "6"))
PIPE_DEPTH = int(os.environ.get("K_PIPE_DEPTH", "2"))
# K_OFFBIG exp units per (b,h) are computed on the DVE via the Schraudolph
# bit-trick in fp16 space (one tensor_scalar producing int16 exponent bits,
# bitcast to fp16) instead of the Scalar engine, balancing the two engines.
OFF_BIG = int(os.environ.get("K_OFFBIG", "6"))
# fp16-space Schraudolph: bits16 = int16(x*A16 + B16) viewed as fp16
SCH_A16 = float(2**10 / np.log(2)) * SCALE
SCH_B16 = float(15 * 2**10) - 60.0
PREP_AT = int(os.environ.get("K_PREP_AT", "23"))


def _chunks(a, b, grid):
    out = []
    while a < b:
        nxt = min(b, (a // grid + 1) * grid)
        out.append((a, nxt))
        a = nxt
    return out


def build_nc():
    nc = bacc.Bacc()
    q_ext = nc.declare_dram_parameter("q", [BH_PER_CORE, S, D], F32, isOutput=False)
    k_ext = nc.declare_dram_parameter("k", [BH_PER_CORE, S, D], F32, isOutput=False)
    v_ext = nc.declare_dram_parameter("v", [BH_PER_CORE, S, D], F32, isOutput=False)
    out_ext = nc.declare_dram_parameter("out", [BH_PER_CORE, S, D], F32, isOutput=True)

    with tile.TileContext(nc) as tc:
        with (
            tc.tile_pool(name="const", bufs=1) as const_pool,
            tc.tile_pool(name="io", bufs=2) as io_pool,
            tc.tile_pool(name="bfp", bufs=2) as bf_pool,
            tc.tile_pool(name="tr", bufs=2) as tr_pool,
            tc.tile_pool(name="ep", bufs=E_BUFS) as e_pool,
            tc.tile_pool(name="epi", bufs=2) as epi_pool,
            tc.tile_pool(name="ps", bufs=ST_BUFS, space="PSUM") as ps_pool,
            tc.tile_pool(name="po", bufs=2, space="PSUM") as po_pool,
            tc.tile_pool(name="pp", bufs=2, space="PSUM") as pp_pool,
        ):
            if LAYOUT == "A":
                prep_alloc = lambda shape, dt: ps_pool.tile(shape, dt, tag="st", name="prep")
                tp_alloc = lambda shape, dt: ps_pool.tile(shape, dt, tag="st", name="tp")
            elif LAYOUT == "B":
                prep_alloc = lambda shape, dt: ps_pool.tile(shape, dt, tag="st", name="prep")
                tp_alloc = lambda shape, dt: po_pool.tile(shape, dt, tag="ot", name="tp")
            else:
                prep_alloc = lambda shape, dt: pp_pool.tile(shape, dt, tag="pp", name="prep")
                tp_alloc = lambda shape, dt: pp_pool.tile(shape, dt, tag="pp", name="tp")
            ident = const_pool.tile([P, P], F16)
            masks.make_identity(nc, ident)
            ident32 = const_pool.tile([D + 1, D + 1], F32)
            masks.make_identity(nc, ident32)
            # touch Exp once so the ACT table load overlaps the first DMAs
            warm = const_pool.tile([P, 1], F32)
            nc.vector.memset(warm, 0.0)
            nc.scalar.activation(out=warm, in_=warm,
                                 func=mybir.ActivationFunctionType.Exp)
            # keep-mask for the diagonal score tile: 1 where j_local <= i_local
            trimask = const_pool.tile([P, P], F16)
            masks.make_upper_triangular(nc, trimask, val=1.0, diag=True)

            def prep(bh):
                """Load Q,K,V; cast to fp16; PE-transpose Q,K.

                qt: [128, S] with QT duplicated on both partition halves;
                kt: [128, NT/2, 128] with even key-tiles on partitions 0-63
                and odd tiles on 64-127.  Adjacent jb score matmuls then hit
                different PE row-groups and run concurrently on hardware.
                Work is pipelined in 4-tile groups so casts/transposes start
                before the full tensors arrive.
                """
                qt = tr_pool.tile([P, S], F16, tag="qt", name="qt")
                kt = tr_pool.tile([P, NT // 2, P], F16, tag="kt", name="kt")
                q_view = q_ext[bh].rearrange("(t p) d -> p t d", p=P)
                k_view = k_ext[bh].rearrange("(t p) d -> p t d", p=P)
                for g in range(NT // 8):
                    k_sb = io_pool.tile([P, 8, D], F32, tag="k_sb", name="k_sb")
                    nc.sync.dma_start(out=k_sb, in_=k_view[:, 8 * g:8 * g + 8, :])
                    k_bf = bf_pool.tile([P, 8, D], F16, tag="k_bf", name="k_bf")
                    nc.gpsimd.tensor_copy(k_bf, k_sb)
                    k_tp = prep_alloc([P, 4 * P], F16)
                    for u in range(4):
                        nc.tensor.transpose(
                            k_tp[:, u * P:(u + 1) * P],
                            k_bf[:, 2 * u:2 * u + 2, :], ident)
                    nc.vector.tensor_copy(kt[:, g * 4:(g + 1) * 4, :], k_tp)
                for g in range(NT // 4):
                    q_sb = io_pool.tile([P, 4, D], F32, tag="q_sb", name="q_sb")
                    nc.sync.dma_start(out=q_sb, in_=q_view[:, 4 * g:4 * g + 4, :])
                    q_bf = bf_pool.tile([P, 4, D], F16, tag="q_bf", name="q_bf")
                    nc.gpsimd.tensor_copy(q_bf, q_sb)
                    q_tp = prep_alloc([D, 4 * P], F16)
                    for u in range(4):
                        nc.tensor.transpose(
                            q_tp[:, u * P:(u + 1) * P], q_bf[:, u, :], ident)
                    sl = slice(g * 4 * P, (g + 1) * 4 * P)
                    nc.vector.tensor_copy(qt[:D, sl], q_tp)
                    # duplicate onto partitions 64-127 for row-group packing
                    # (gpsimd cannot read PSUM, so copy from the SBUF half)
                    nc.gpsimd.tensor_copy(qt[D:, sl], qt[:D, sl])
                v_sb = io_pool.tile([P, NT, D], F32, tag="v_sb", name="v_sb")
                nc.sync.dma_start(
                    out=v_sb, in_=v_ext[bh].rearrange("(t p) d -> p t d", p=P))
                vp = bf_pool.tile([P, NT, D + 1], F16, tag="vp", name="vp")
                nc.gpsimd.tensor_copy(vp[:, :, :D], v_sb)
                nc.gpsimd.memset(vp[:, :, D:], 1.0)
                return qt, kt, vp

            preps = {0: prep(0)}
            for bh in range(BH_PER_CORE):
                qt, kt, vp = preps.pop(bh)
                # ---- main loop: i-chunk major, software-pipelined units ----
                # unit = (ci, jb): score block ST[j, i] for
                #   i in [max(ci*CW, jb*P), (ci+1)*CW), j in [jb*P, (jb+1)*P)
                units = []
                for ci in range(NC_CHUNK):
                    c0 = ci * CW
                    for jb in range(((ci + 1) * CW) // P):
                        units.append((ci, jb, max(c0, jb * P), (ci + 1) * CW))

                ots = {}     # (ci, cell0) -> psum accumulator [D+1, 512]
                stage = []   # pipelined: [(unit, st_tile, e_tile), ...]
                pending = []  # completed cells awaiting drain

                # spread the DVE-offloaded units over units wide enough that
                # their PV chunks stay >= 256 (f32r full-rate)
                # offloadable: wide enough for full-rate PV, and never a
                # dominant share of any query row's softmax mass (rows in
                # [jb*P, ...) get 1/(jb+1) of their mass from key-tile jb)
                cands = [i for i, (ci_, jb_, u0_, u1_) in enumerate(units)
                         if u1_ - u0_ >= 384 and (ci_ == 1 or jb_ >= 3)]
                _soff = int(os.environ.get("K_OFF_SHIFT", "1"))
                off_set = set(
                    cands[(round(i * len(cands) / OFF_BIG) + _soff) % len(cands)]
                    for i in range(OFF_BIG)) if OFF_BIG else set()

                def flush_pv(ci, jb, u0, u1, e_sb):
                    for (a, b) in _chunks(u0, u1, 512):
                        # each 512-wide output cell has its own accumulator;
                        # cell_last is the last key-tile writing it
                        cell0 = (a // 512) * 512
                        cell_last = (cell0 + 511) // P
                        nc.tensor.matmul(
                            ots[ci, cell0][:, a - cell0:b - cell0],
                            vp[:, jb, :], e_sb[:, a - u0:b - u0],
                            start=(jb == 0), stop=(jb == cell_last))
                        if jb == cell_last:
                            pending.append((ci, cell0, ots.pop((ci, cell0))))

                for uidx, (ci, jb, u0, u1) in enumerate(units):
                    if jb == 0:
                        for cell0 in range(ci * CW, (ci + 1) * CW, 512):
                            ots[ci, cell0] = po_pool.tile(
                                [D + 1, 512], F32, tag="ot", name=f"ot{cell0}")
                    w = u1 - u0
                    st = ps_pool.tile([P, CW], F32, tag="st")
                    half = (jb % 2) * D
                    for (a, b) in _chunks(0, w, 512):
                        nc.tensor.matmul(
                            st[:, a:b], kt[half:half + D, jb // 2, :],
                            qt[half:half + D, u0 + a:u0 + b])
                    if uidx in off_set:
                        ei = e_pool.tile([P, CW], mybir.dt.int16, tag="e",
                                         name="ei")
                        nc.vector.tensor_scalar(
                            out=ei[:, :w], in0=st[:, :w],
                            scalar1=SCH_A16, scalar2=SCH_B16,
                            op0=mybir.AluOpType.mult, op1=mybir.AluOpType.add)
                        e_sb = ei.bitcast(F16)
                    else:
                        e_sb = e_pool.tile([P, CW], F16, tag="e")
                        nc.scalar.activation(
                            out=e_sb[:, :w], in_=st[:, :w],
                            func=mybir.ActivationFunctionType.Exp, scale=SCALE)
                    if u0 == jb * P:  # diagonal tile: causal mask
                        nc.vector.tensor_mul(
                            e_sb[:, :P], e_sb[:, :P], trimask)
                    stage.append((ci, jb, u0, u1, e_sb))
                    if len(stage) > PIPE_DEPTH:
                        flush_pv(*stage.pop(0))
                    if len(pending) > 1:
                        _epilogue_cell(nc, *pending.pop(0), epi_pool,
                                       tp_alloc, ident32, out_ext, bh)
                    if uidx == PREP_AT and bh + 1 < BH_PER_CORE:
                        preps[bh + 1] = prep(bh + 1)
                    if jb == ((ci + 1) * CW) // P - 1:
                        # chunk finished: flush the pipeline and drain cells
                        while stage:
                            flush_pv(*stage.pop(0))
                        while pending:
                            _epilogue_cell(nc, *pending.pop(0), epi_pool,
                                           tp_alloc, ident32, out_ext, bh)

    nc.compile()
    return nc


def _epilogue_cell(nc, ci, cell0, ot, epi_pool, tp_alloc, ident32, out_ext,
                   bh):
    """Drain one completed 512-wide O^T cell: copy out of PSUM, transpose
    back to [i, c] tiles, normalize by the accumulated denominator, DMA out.
    """
    ntile = 512 // P  # 4 query tiles
    ot_sb = epi_pool.tile([D + 1, 512], F32, tag="ot_sb", name="ot_sb")
    nc.vector.tensor_copy(ot_sb, ot)
    o_sb = epi_pool.tile([P, ntile, D], F32, tag="o_sb", name="o_sb")
    rcp = epi_pool.tile([P, ntile], F32, tag="rcp", name="rcp")
    # inner dim padded to 66 to keep per-transpose offsets regular
    tp = tp_alloc([P, 4, D + 2], F32)
    for u in range(ntile):
        nc.tensor.transpose(
            tp[:, u, :D + 1], ot_sb[:, u * P:(u + 1) * P], ident32)
    nc.vector.reciprocal(out=rcp, in_=tp[:, :, D])
    rcp_b = bass.AP(tensor=rcp.tensor, offset=rcp.offset,
                    ap=[rcp.ap[0], rcp.ap[1], [0, D]])
    nc.vector.tensor_tensor(
        out=o_sb, in0=tp[:, :, :D], in1=rcp_b, op=mybir.AluOpType.mult)
    nc.sync.dma_start(
        out=out_ext[bh, cell0:cell0 + 512].rearrange("(t p) d -> p t d", p=P),
        in_=o_sb)


_CACHE = {}


def _get_runner():
    """Build + compile once; return a cached jitted 8-core runner."""
    if "runner" in _CACHE:
        return _CACHE["runner"]

    import jax
    from jax.sharding import Mesh, PartitionSpec
    from jax.experimental.shard_map import shard_map
    from concourse import bass2jax
    from concourse.bass2jax import _bass_exec_p, partition_id_tensor
    import concourse.mybir as _mybir

    nc = build_nc()
    bass2jax.install_neuronx_cc_hook()

    partition_name = nc.partition_id_tensor.name if nc.partition_id_tensor else None
    in_names, out_names, out_avals = [], [], []
    for alloc in nc.m.functions[0].allocations:
        if not isinstance(alloc, _mybir.MemoryLocationSet):
            continue
        name = alloc.memorylocations[0].name
        if alloc.kind == "ExternalInput":
            if name != partition_name:
                in_names.append(name)
        elif alloc.kind == "ExternalOutput":
            shape = tuple(alloc.tensor_shape)
            dtype = _mybir.dt.np(alloc.dtype)
            out_names.append(name)
            out_avals.append(jax.core.ShapedArray(shape, dtype))
    n_params = len(in_names)
    all_names = list(in_names) + list(out_names)
    if partition_name is not None:
        all_names.append(partition_name)

    def _body(*args):
        operands = list(args)
        if partition_name is not None:
            operands.append(partition_id_tensor())
        outs = _bass_exec_p.bind(
            *operands,
            out_avals=tuple(out_avals),
            in_names=tuple(all_names),
            out_names=tuple(out_names),
            lowering_input_output_aliases=(),
            sim_require_finite=True,
            sim_require_nnan=True,
            nc=nc,
        )
        return tuple(outs)

    devices = jax.devices()[:N_CORES]
    mesh = Mesh(np.asarray(devices), ("core",))
    n_outs = len(out_names)
    in_specs = (PartitionSpec("core"),) * (n_params + n_outs)
    out_specs = (PartitionSpec("core"),) * n_outs
    sharded = jax.jit(shard_map(
        _body, mesh=mesh, in_specs=in_specs, out_specs=out_specs,
        check_rep=False))

    runner = {
        "fn": sharded,
        "in_names": in_names,
        "out_names": out_names,
        "out_avals": out_avals,
        "mesh": mesh,
    }
    _CACHE["runner"] = runner
    return runner


def _shard(x):
    """[B, H, S, D] -> concatenated per-core [(N_CORES*BH_PER_CORE), S, D]."""
    return np.ascontiguousarray(x.reshape(B * H, S, D))


def kernel(q, k, v):
    q = np.asarray(q, dtype=np.float32)
    k = np.asarray(k, dtype=np.float32)
    v = np.asarray(v, dtype=np.float32)
    r = _get_runner()
    ins = {"q": _shard(q), "k": _shard(k), "v": _shard(v)}
    concat_in = [ins[name] for name in r["in_names"]]
    zeros = [np.zeros((N_CORES * av.shape[0],) + av.shape[1:], av.dtype)
             for av in r["out_avals"]]
    outs = r["fn"](*concat_in, *zeros)
    out = np.asarray(outs[r["out_names"].index("out")])
    return out.reshape(B, H, S, D)

